# revision 1
# baseline (speedup 1.0000x reference)
"""Trainium2 Bass kernel for nn_BitwiseLinear (8 NeuronCores, SPMD).

Reference semantics (B=32768, IN=OUT=1024):
    out = in_scale * weight_scale * (sign(x) @ sign(weight * gate_mask).T + bias)
    gate_mask = (sign(gate)+1)/2; in_scale = mean|x| per row; weight_scale = mean|w| per out.

Identities used:
    sign(weight * gate_mask) == sign(weight) * (gate >= 0)  (gate==0 -> mask 0.5 -> sign(w))
    out = sum|x|_row * (signmm + bias) * ws_eff,  ws_eff = sum|w|_row * 2^-20

Sharding: data-parallel on batch across the 8 cores (x/out sharded 4096 rows
per core), weight/gate/bias replicated, no collectives.

Per core (v5 — minimal-pass weight prep, fused epilogue):
  prep: weight chunks split across BOTH DMA rings (even chunks head the sync
        HWDGE queue, odd chunks the gpsimd SWDGE queue) so the o-lo half
        lands at aggregate rate; RAW f32 chunks are PE-transposed as they
        arrive and a single ACT Sign per (K-chunk, o-half) binarizes straight
        from PSUM into the fp8 wtq moving operand (no separate evac pass);
        |w| row sums ride the DVE queue head, ws_row scaling on DVE.
  loop over 32 x-tiles (1MB group DMAs):
        per half: 4 PE transposes -> PSUM -> ACT Sign -> fp8 sign(x).T;
        transpose/sign emission interleaved between the DR matmul groups two
        tiles ahead; 8 fp8 DoubleRow matmuls (K=256) accumulate the 1024-deep
        contraction into two PSUM banks; |x| row sums alternate ACT
        Abs+accum / DVE tensor_reduce; epilogue fused in one DVE
        scalar_tensor_tensor per bank: (psum * is_raw[b]) * ws_bcast[o]
        straight into the out group buffer; group DMAs issue immediately.
  Runtime specialization: bias matmuls dropped when bias is all-zero and the
  gate path dropped when gate >= 0 everywhere (checked against the actual
  inputs; other variants compile lazily and remain correct).
"""

import numpy as np

import concourse.bacc as bacc
import concourse.mybir as mybir
import concourse.tile as tile
from concourse import masks
from concourse.bass_utils import run_bass_kernel_spmd

B, IN, OUT = 32768, 1024, 1024
NCORES = 8
BSH = B // NCORES            # 4096 rows per core
P = 128                      # partitions
NT = BSH // P                # 32 x-tiles per core
KC = IN // P                 # 8 contraction chunks of 128
NPAIR = KC // 2              # 4 DoubleRow K-pairs (256 each)
NCH = 512                    # matmul moving free-dim (one PSUM bank of f32)
F32 = mybir.dt.float32
I32 = mybir.dt.int32
BF16 = mybir.dt.bfloat16
FP8 = mybir.dt.float8e4
WS_SCALE = float(2.0 ** -20)  # 1/(1024*1024): folds both mean divisors

_CACHE: dict = {}


def _build(with_bias=True, with_gate=True):
    nc = bacc.Bacc("TRN2", target_bir_lowering=False, debug=False,
                   num_devices=NCORES)

    x_ext = nc.declare_dram_parameter("x", [BSH, IN], BF16, isOutput=False)
    w_ext = nc.declare_dram_parameter("weight", [OUT, IN], BF16, isOutput=False)
    g_ext = nc.declare_dram_parameter("gate", [OUT, IN], F32, isOutput=False)
    b_ext = nc.declare_dram_parameter("bias", [1, OUT], F32, isOutput=False)
    o_ext = nc.declare_dram_parameter("out", [BSH, OUT], BF16, isOutput=True)

    x_ap = x_ext.ap()
    w_ap = w_ext.ap()
    g_ap = g_ext.ap()
    b_ap = b_ext.ap()
    o_ap = o_ext.ap()

    ACT = mybir.ActivationFunctionType
    ALU = mybir.AluOpType
    AX = mybir.AxisListType
    DR = mybir.MatmulPerfMode.DoubleRow

    with tile.TileContext(nc) as tc:
        with tc.tile_pool(name="const", bufs=1) as cp:
            ident_f32 = cp.tile([P, P], F32)
            ident_bf = cp.tile([P, P], BF16)
            ones_f8 = cp.tile([1, P], FP8)
            ones_f32 = cp.tile([1, P], F32)
            zbias = cp.tile([P, 1], F32)

            # persistent prepped weights
            # pair j holds binarized wT chunks 2j (at [:, :OUT]) and 2j+1
            wtq = [cp.tile([P, 2 * OUT], FP8, tag=f"wtq{j}", name=f"wtq{j}")
                   for j in range(NPAIR)]
            bias_f8 = cp.tile([1, OUT], FP8)      # raw bias (fp8) added pre-scale
            ws_bcast = cp.tile([P, OUT], F32)     # ws * 2^-20 broadcast over partitions

            with tc.tile_pool(name="wraw", bufs=1) as wraw_pool, \
                 tc.tile_pool(name="wkeep", bufs=1) as wk, \
                 tc.tile_pool(name="wsgn", bufs=2) as wsgn_pool, \
                 tc.tile_pool(name="xin", bufs=8) as xin_pool, \
                 tc.tile_pool(name="xbt", bufs=6) as xbt_pool, \
                 tc.tile_pool(name="opair", bufs=3) as opair_pool, \
                 tc.tile_pool(name="sc", bufs=14) as sc_pool, \
                 tc.tile_pool(name="gscr", bufs=3) as gscr_pool, \
                 tc.tile_pool(name="pst", bufs=3, space="PSUM") as pst_pool, \
                 tc.tile_pool(name="pso", bufs=5, space="PSUM") as pso_pool:
                G = 2                 # tiles per x/out DMA group (1 MiB)
                NG = NT // G
                xts = [None] * NT
                xbts = [None] * NT
                is_raws = [None] * NT
                out_groups = [None] * NG
                staged = {}

                masks.make_identity(nc, ident_f32[:])
                masks.make_identity(nc, ident_bf[:])
                nc.gpsimd.memset(ones_f8[:], 1.0)
                nc.gpsimd.memset(ones_f32[:], 1.0)
                nc.gpsimd.memset(zbias[:], 0.0)
                # dummy tiny ACTIVATE: walrus places the ~1.3us
                # ACT_TABLE_LOAD before the FIRST activation -- fire it here
                # in the idle preamble instead of on the weight-sign chain
                warm = wk.tile([P, 1], F32)
                nc.scalar.activation(warm[:], zbias[:], ACT.Sign,
                                     bias=zbias[:])

                # ---------------- DMA plan -------------------------------
                # even weight chunks head the sync ring, odd chunks the
                # gpsimd ring -> the o-lo half (chunks 0-3) lands at the
                # aggregate rate of both rings; x groups queue behind on
                # sync and stream for the rest of the kernel.
                wts = [wraw_pool.tile([P, IN], BF16, tag=f"wch{t}",
                                      name=f"wch{t}") for t in range(KC)]
                gts = []
                if with_gate:
                    gts = [wraw_pool.tile([P, IN], F32, tag=f"gch{t}",
                                          name=f"gch{t}") for t in range(KC)]

                def w_dma(t):
                    # all weights ride the sync HWDGE ring: the SWDGE ring
                    # loses SDMA arbitration badly when HWDGE is loaded
                    nc.sync.dma_start(wts[t][:], w_ap[t * P:(t + 1) * P, :])
                    if with_gate:
                        nc.sync.dma_start(gts[t][:], g_ap[t * P:(t + 1) * P, :])

                def stage_front_group(m):
                    """x-group DMA only; transposes/signs emitted separately."""
                    xtg = xin_pool.tile([P, G * IN], BF16, tag="xtg",
                                        name=f"xtg{m}")
                    nc.sync.dma_start(
                        xtg[:].rearrange("p (t i) -> p t i", t=G),
                        x_ap[m * G * P:(m + 1) * G * P, :].rearrange(
                            "(t p) i -> p t i", p=P))
                    for t in range(G):
                        xts[G * m + t] = xtg[:, t * IN:(t + 1) * IN]

                for t in range(KC):
                    w_dma(t)
                for m in range(4):
                    stage_front_group(m)
                ws_cols = wk.tile([P, KC], F32)   # per-o |w| row sums, col t
                bias_sb = wk.tile([1, OUT], F32)
                ws_row = wk.tile([1, OUT], F32)
                if with_bias:
                    nc.sync.dma_start(bias_sb[:], b_ap[:, :])
                    nc.vector.tensor_copy(bias_f8[:], bias_sb[:])

                # |w| row sums at the head of the DVE queue (chunk-gated)
                for t in range(KC):
                    nc.vector.tensor_reduce(ws_cols[:, t:t + 1], wts[t][:],
                                            axis=AX.X, op=ALU.add,
                                            apply_absolute_value=True)

                # ---------------- weight prep ----------------------------
                # per (c, h): 4 PE transposes of the RAW f32 chunks into one
                # PSUM bank, then a single ACT Sign binarizes PSUM -> wtq
                # (fp8) directly.  Half h only needs chunks 4h..4h+3.
                def w_piece(c):
                    # all 8 o-chunks of wT K-chunk c in one 1-bank bf16 psum
                    ps_w = pso_pool.tile([P, IN], BF16, tag="ps_o",
                                         name=f"ps_w{c}")
                    for t in range(KC):
                        nc.tensor.transpose(
                            ps_w[:, t * P:(t + 1) * P],
                            wts[t][:, c * P:(c + 1) * P],
                            ident_bf[:])
                    dst = wtq[c // 2][:, (c % 2) * OUT:(c % 2) * OUT + OUT]
                    nc.scalar.activation(dst, ps_w[:], ACT.Sign,
                                         bias=zbias[:])

                def w_half(h):
                    for c in range(KC):
                        ps_w = pso_pool.tile([P, NCH], BF16, tag="ps_o",
                                             name=f"ps_w{h}_{c}")
                        for tt in range(4):
                            t = h * 4 + tt
                            nc.tensor.transpose(
                                ps_w[:, tt * P:(tt + 1) * P],
                                wts[t][:, c * P:(c + 1) * P],
                                ident_bf[:])
                        dst = wtq[c // 2][:, (c % 2) * OUT + h * NCH:
                                          (c % 2) * OUT + (h + 1) * NCH]
                        if not with_gate:
                            nc.scalar.activation(dst, ps_w[:], ACT.Sign,
                                                 bias=zbias[:])
                        else:
                            ps_g = pso_pool.tile([P, NCH], F32, tag="ps_o",
                                                 name=f"ps_g{h}_{c}")
                            for tt in range(4):
                                t = h * 4 + tt
                                nc.tensor.transpose(
                                    ps_g[:, tt * P:(tt + 1) * P],
                                    gts[t][:, c * P:(c + 1) * P],
                                    ident_f32[:])
                            sgn = wsgn_pool.tile([P, NCH], BF16, tag="wsgn")
                            nc.scalar.activation(sgn[:], ps_w[:], ACT.Sign,
                                                 bias=zbias[:])
                            # (gate>=0) * sign(w) fused on DVE
                            nc.vector.scalar_tensor_tensor(
                                dst, ps_g[:], 0.0, sgn[:],
                                op0=ALU.is_ge, op1=ALU.mult)

                # stage a whole x group: 16 PE transposes in one burst (one
                # transpose-mode switch), then the 4 ACT Signs
                def x_group_stage(m):
                    pend = []
                    for t in range(G):
                        it = G * m + t
                        if it >= NT:
                            return
                        xbts[it] = xbt_pool.tile([P, IN], FP8, tag="xbT",
                                                 name="xbT")
                        xt = xts[it]
                        # [P, IN] bf16 = 2KB/partition = ONE psum bank
                        ps_t = pst_pool.tile([P, IN], BF16, tag="ps_t")
                        for c in range(KC):
                            nc.tensor.transpose(
                                ps_t[:, c * P:(c + 1) * P],
                                xt[:, c * P:(c + 1) * P],
                                ident_bf[:])
                        pend.append((it, ps_t))
                    for it, ps_t in pend:
                        nc.scalar.activation(xbts[it][:], ps_t[:], ACT.Sign,
                                             bias=zbias[:])

                def emit_reduce(which):
                    if which >= NT or is_raws[which] is not None:
                        return
                    is_raw = sc_pool.tile([P, 1], F32, tag="is_raw",
                                          name=f"is_raw{which}")
                    # last tiles' reduces ride ACT (idle by then) so the DVE
                    # STT drain at the end has no reduce backlog
                    if which % 2 == 0 or which >= NT - 4:
                        scr = gscr_pool.tile([P, IN], BF16, tag="gscr",
                                             name="xabs_scr")
                        nc.scalar.activation(scr[:], xts[which], ACT.Abs,
                                             bias=zbias[:], accum_out=is_raw[:])
                    else:
                        nc.vector.tensor_reduce(
                            is_raw[:], xts[which], axis=AX.X,
                            op=ALU.add, apply_absolute_value=True)
                    is_raws[which] = is_raw

                # prologue emission: all weight pieces first (x data arrives
                # after the weights anyway; x work at the PE/ACT queue heads
                # would only head-of-line-block the weight chain).  The first
                # tiles' |x| reduces go right after their signs: is_raw(0)
                # gates the first STT, which in turn frees the PSUM banks.
                if with_gate:
                    w_half(0)
                    w_half(1)
                else:
                    for c in range(KC):
                        w_piece(c)
                x_group_stage(0)
                emit_reduce(0)
                emit_reduce(1)
                x_group_stage(1)
                emit_reduce(2)
                emit_reduce(3)

                # ws_row[0, o] = sum_i |w[o,i]| * 2^-20 via 8 tiny PE
                # transposes; scale + broadcast handled off the ACT queue.
                for half in range(2):
                    ps_row = pso_pool.tile([1, NCH], F32, tag="ps_o",
                                           name=f"ps_row{half}")
                    for tt in range(4):
                        t = half * 4 + tt
                        nc.tensor.transpose(ps_row[0:1, tt * P:(tt + 1) * P],
                                            ws_cols[:, t:t + 1], ident_f32[:])
                    nc.vector.tensor_scalar(
                        ws_row[:, half * NCH:(half + 1) * NCH], ps_row[:],
                        WS_SCALE, None, op0=ALU.mult)

                # broadcast ws_row across partitions with a K=1 matmul
                for n in range(OUT // NCH):
                    ps_bc = pso_pool.tile([P, NCH], F32, tag="ps_o",
                                          name=f"ps_bc{n}")
                    nc.tensor.matmul(ps_bc[:], ones_f32[:],
                                     ws_row[:, n * NCH:(n + 1) * NCH])
                    nc.vector.tensor_copy(ws_bcast[:, n * NCH:(n + 1) * NCH],
                                          ps_bc[:])

                for pre in range(2 * G):
                    emit_reduce(pre)

                NPRE = 4          # groups DMA'd ahead (4 MiB)

                for it in range(NT):
                    m, t = divmod(it, G)
                    if t == 0:
                        out_groups[m] = opair_pool.tile([P, G * OUT], BF16,
                                                        tag="og", name=f"og{m}")
                        if m + NPRE < NG:
                            stage_front_group(m + NPRE)
                        for t2 in range(G):
                            emit_reduce(it + 2 * G + t2)
                        x_group_stage(m + 2)
                    xb = xbts[it]
                    ps_os = []
                    for n in range(OUT // NCH):
                        ps_os.append(pso_pool.tile([P, NCH], F32, tag="ps_o",
                                                   name=f"ps_o{n}"))
                    for j in range(NPAIR):
                        xp = xb[:, j * 2 * P:(j + 1) * 2 * P].rearrange(
                            "p (two m) -> p two m", two=2)
                        wq = wtq[j][:].rearrange("p (two o) -> p two o", two=2)
                        for n in range(OUT // NCH):
                            nc.tensor.matmul(
                                ps_os[n][:],
                                xp,
                                wq[:, :, n * NCH:(n + 1) * NCH],
                                start=(j == 0),
                                stop=(not with_bias and j == NPAIR - 1),
                                perf_mode=DR)
                    dst = out_groups[m][:, t * OUT:(t + 1) * OUT]
                    for n in range(OUT // NCH):
                        if with_bias:
                            nc.tensor.matmul(ps_os[n][:], ones_f8[:],
                                             bias_f8[:, n * NCH:(n + 1) * NCH],
                                             start=False, stop=True)
                        # fused (psum * is_raw[b]) * ws_bcast[o] -> group buf
                        nc.vector.scalar_tensor_tensor(
                            dst[:, n * NCH:(n + 1) * NCH], ps_os[n][:],
                            is_raws[it][:],
                            ws_bcast[:, n * NCH:(n + 1) * NCH],
                            op0=ALU.mult, op1=ALU.mult)
                    if m == NG - 1:
                        # last group: per-tile DMAs split across both (idle)
                        # rings so the final drain overlaps the last tile
                        eng = nc.gpsimd if t == 0 else nc.sync
                        eng.dma_start(o_ap[it * P:(it + 1) * P, :], dst)
                    elif t == G - 1:
                        # final groups alternate across both rings
                        if m >= NG - 4:
                            eng = nc.sync if m % 2 else nc.gpsimd
                        else:
                            eng = nc.gpsimd
                        eng.dma_start(
                            o_ap[m * G * P:(m + 1) * G * P, :].rearrange(
                                "(u p) o -> p u o", p=P),
                            out_groups[m][:].rearrange("p (u o) -> p u o", u=G))

    nc.compile()
    return nc


def _get_nc(with_bias, with_gate):
    key = f"nc{int(with_bias)}{int(with_gate)}"
    if key not in _CACHE:
        _CACHE[key] = _build(with_bias, with_gate)
    return _CACHE[key]


def run(x, weight, gate, bias, trace=False):
    # gate >= 0 everywhere makes the gate mask exactly 1 (sign(g)+1)/2 with
    # g==0 -> 0.5, and sign(w*0.5) == sign(w)); skip it entirely then.
    nc = _get_nc(bool(np.any(np.asarray(bias))),
                 not bool(np.all(np.asarray(gate) >= 0.0)))
    bf16 = mybir.dt.np(BF16)
    x = np.ascontiguousarray(np.asarray(x, dtype=np.float32).astype(bf16))
    weight = np.ascontiguousarray(np.asarray(weight, dtype=np.float32).astype(bf16))
    gate = np.ascontiguousarray(np.asarray(gate, dtype=np.float32))
    bias = np.ascontiguousarray(np.asarray(bias, dtype=np.float32)).reshape(1, OUT)
    in_maps = [
        {"x": x[i * BSH:(i + 1) * BSH], "weight": weight, "gate": gate, "bias": bias}
        for i in range(NCORES)
    ]
    res = run_bass_kernel_spmd(nc, in_maps, core_ids=list(range(NCORES)), trace=trace)
    out = np.concatenate([res.results[i]["out"] for i in range(NCORES)],
                     axis=0).astype(np.float32)
    return out, res


def kernel(x, weight, gate, bias):
    out, _ = run(x, weight, gate, bias, trace=False)
    return out

